# revision 34
# baseline (speedup 1.0000x reference)
"""TRN2 Bass kernel: MultiHeadSelfAttention (B=4, S=2048, D=1024, H=16, DK=64).

Sharding: 8 cores = 4 batches x 2 head-groups (8 heads each).

Key optimization vs the dense version: the padding mask kills ~half the keys
(exp(-1e6) == 0 exactly in f32) and ~half the queries (output is multiplied
by the query mask), so the host compacts each batch to its valid rows
(max 1044 for this distribution) padded to SV=1152. All attention work
(QK, softmax, PV) shrinks ~3.2x and the projections ~1.8x, exactly.

Per core: QK in f32r (TF32), softmax via one wide reduce_max (negated) +
one wide exp(bias=-max) -> bf16 P, P^T via DMA-transpose, PV with [V|1]
stationary -> [O^T; denom], 1/denom broadcast, normalization on gpsimd,
output projection from O^T, partial Y out. Host sums the two head-group
partials, applies abs, and scatters to valid positions.

The (qb, head) stream is software-pipelined: PV/output-projection for
head j runs while QK/softmax for head j+LAG streams, so PE never waits
on the DVE->Act->DMA softmax chain.
"""

import os
import numpy as np

B, S, D, H, DK = 4, 2048, 1024, 16, 64
HG = 2            # head groups (tensor-parallel)
HL = H // HG      # heads per core = 8
DH = HL * DK      # 512 per-core head width
KT = D // 128     # 8 contraction tiles
SV = 1152         # padded valid-row count (max valid ~1044 for this dist)
SVC = 1056        # trimmed compute width (>= max valid count, mult of 32)
NT = SV // 128    # 9 tiles of 128 (q tiles == k chunks)
QBS = (512, 512, 128)   # q block widths (sum == SV)
QBC = (512, 512, 32)    # q block valid widths (sum == SVC)
LAG = 3           # software pipeline depth in heads

_cache = {}


def _build():
    from concourse import bacc
    import concourse.mybir as mybir
    import concourse.tile as tile

    f32 = mybir.dt.float32
    f32r = mybir.dt.float32r
    bf16 = mybir.dt.bfloat16
    Exp = mybir.ActivationFunctionType.Exp
    AX = mybir.AxisListType.X

    nc = bacc.Bacc("TRN2", target_bir_lowering=False, debug=False, num_devices=8)

    xT_d = nc.dram_tensor("xT", [D, SV], f32, kind="ExternalInput")
    wq_d = nc.dram_tensor("wq", [D, DH], f32, kind="ExternalInput")
    wk_d = nc.dram_tensor("wk", [D, DH], f32, kind="ExternalInput")
    wv_d = nc.dram_tensor("wv", [D, DH], f32, kind="ExternalInput")
    wo_d = nc.dram_tensor("wo", [DH, D], f32, kind="ExternalInput")
    y_d = nc.dram_tensor("y", [SV, D], f32, kind="ExternalOutput")

    with tile.TileContext(nc) as tc:
        with (
            tc.tile_pool(name="persist", bufs=1) as pp,
            tc.tile_pool(name="psS", bufs=2, space="PSUM") as psS,
            tc.tile_pool(name="psSm", bufs=2, space="PSUM") as psSm,
            tc.tile_pool(name="ptbp", bufs=LAG + 1) as ptbp,
            tc.tile_pool(name="pexp", bufs=3) as pexp,
            tc.tile_pool(name="stats", bufs=2) as st,
            tc.tile_pool(name="oTp", bufs=2) as oTp,
            tc.tile_pool(name="yp", bufs=2) as yp,
            tc.tile_pool(name="ph1x", bufs=1) as px,
            tc.tile_pool(name="ph1w", bufs=2) as pw,
            tc.tile_pool(name="ph1wv", bufs=1) as pwv,
        ):
            qT = pp.tile([128, 4, SV], f32r, tag="qT")
            kT = pp.tile([128, 4, SV], f32r, tag="kT")
            # V with a ones column per head: blocks of 66 = [V_h(64) | 1 | pad]
            v_sb = pp.tile([128, NT, HL, 66], bf16, tag="v")
            nc.gpsimd.memset(v_sb[:, :, :, 64:65], 1.0)
            wor = pp.tile([128, 4, D], f32r, tag="wor")

            # ---- input loads ----
            xr = px.tile([128, KT, SV], f32r, tag="xr")
            nc.gpsimd.dma_start(xr[:], xT_d.rearrange("(t p) s -> p t s", p=128))
            wvr = pwv.tile([128, KT, DH], f32r, tag="wvr")
            wvr_loaded = [False]

            def load_wv_wo():
                nc.gpsimd.dma_start(
                    wvr[:], wv_d.rearrange("(t p) n -> p t n", p=128))
                nc.gpsimd.dma_start(
                    wor[:], wo_d.rearrange("(t p) n -> p t n", p=128))

            def proj_qk(w_d, dst, p):
                wchs = pw.tile([128, KT, 128], f32r, tag="wch")
                nc.gpsimd.dma_start(
                    wchs[:],
                    w_d[:, p * 128:(p + 1) * 128].rearrange(
                        "(t p) n -> p t n", p=128),
                )
                ps = psS.tile([128, SV], f32, tag="S")
                n0 = 0
                for nw in (512, 512, 32):
                    for k in range(KT):
                        nc.tensor.matmul(
                            ps[:, n0:n0 + nw],
                            wchs[:, k, :],
                            xr[:, k, n0:n0 + nw],
                            start=(k == 0),
                            stop=(k == KT - 1),
                        )
                    n0 += nw
                nc.vector.tensor_copy(dst[:, p, 0:SVC], ps[:, 0:SVC])

            def proj_v(sc):
                psv = psSm.tile([128, 512], f32, tag="mm")
                for k in range(KT):
                    nc.tensor.matmul(
                        psv[:],
                        xr[:, k, sc * 128:(sc + 1) * 128],
                        wvr[:, k, :],
                        start=(k == 0),
                        stop=(k == KT - 1),
                    )
                nc.scalar.copy(
                    v_sb[:, sc, :, 0:64],
                    psv[:].rearrange("p (h w) -> p h w", w=64),
                )

            # Q/K projections for dh tile 0 up front; tiles 1-3 are emitted
            # inside the attention stream right before the heads needing them
            proj_qk(wq_d, qT, 0)
            proj_qk(wk_d, kT, 0)
            load_wv_wo()

            # ---- attention stream, software-pipelined over (qb, head) ----
            qb_off = [0, 512, 1024]
            oTs = {}

            il_ctr = [0]

            def emit_qk_il(qb, hh, il, ptb):
                p, r0 = hh // 2, (hh % 2) * 64
                i = qb_off[qb] // 128 + il
                sq = psS.tile([128, SV], f32, tag="S")
                n0 = 0
                for nw in (512, 512, 32):
                    nc.tensor.matmul(
                        sq[:, n0:n0 + nw],
                        qT[r0:r0 + DK, p, i * 128:(i + 1) * 128],
                        kT[r0:r0 + DK, p, n0:n0 + nw],
                        start=True,
                        stop=True,
                    )
                    n0 += nw
                nm = st.tile([128, 1], f32, tag="nm")
                nc.vector.tensor_reduce(
                    nm[:], sq[:, 0:SVC], axis=AX,
                    op=mybir.AluOpType.max, negate=True,
                )
                il_ctr[0] += 1
                p_sb = pexp.tile([128, SV], bf16, tag="p")
                # exp in two ops whose read ranges match the QK chunk writes:
                # the next tile reusing this S buffer can start its first
                # chunk as soon as the first exp half has drained.
                nc.scalar.activation(
                    p_sb[:, 0:512], sq[:, 0:512], Exp, bias=nm[:], scale=1.0,
                )
                nc.scalar.activation(
                    p_sb[:, 512:SVC], sq[:, 512:SVC], Exp, bias=nm[:], scale=1.0,
                )
                nc.sync.dma_start(
                    ptb[:, :, il * 128:(il + 1) * 128],
                    p_sb[:],
                    transpose=True,
                )

            def emit_pv_mm(qb, hh, ptb):
                qw = QBC[qb]
                if hh == 0:
                    oTs[qb] = oTp.tile([128, 4, 512], f32r, tag="oT",
                                       name=f"oT{qb}")
                ot_ps = psSm.tile([65, 512], f32, tag="mm")
                for kc in range(NT):
                    kk = 128 if kc < NT - 1 else SVC - 128 * (NT - 1)
                    nc.tensor.matmul(
                        ot_ps[:, 0:qw],
                        v_sb[0:kk, kc, hh, 0:65],
                        ptb[0:kk, kc, 0:qw],
                        start=(kc == 0),
                        stop=(kc == NT - 1),
                    )
                return ot_ps

            def emit_pv_norm(qb, hh, ot_ps):
                p, r0 = hh // 2, (hh % 2) * 64
                qw = QBC[qb]
                rrow = st.tile([1, 512], f32, tag="rrow")
                nc.vector.reciprocal(rrow[:, 0:qw], ot_ps[64:65, 0:qw])
                rb = st.tile([64, 512], f32, tag="rb")
                nc.gpsimd.partition_broadcast(rb[:, 0:qw], rrow[:, 0:qw])
                osb = st.tile([64, 512], f32, tag="osb")
                nc.vector.tensor_copy(osb[:, 0:qw], ot_ps[0:64, 0:qw])
                nc.gpsimd.tensor_tensor(
                    oTs[qb][r0:r0 + 64, p, 0:qw], osb[:, 0:qw],
                    rb[:, 0:qw], op=mybir.AluOpType.mult,
                )

            def emit_op(qb):
                """Output projection for q block qb."""
                qw = QBS[qb]
                oT = oTs[qb]
                for il in range(qw // 128):
                    i = qb_off[qb] // 128 + il
                    y_sb = yp.tile([128, D], f32, tag="y")
                    for half in range(2):
                        yq = psSm.tile([128, 512], f32, tag="mm")
                        for pp_ in range(4):
                            nc.tensor.matmul(
                                yq[:],
                                oT[:, pp_, il * 128:(il + 1) * 128],
                                wor[:, pp_, half * 512:(half + 1) * 512],
                                start=(pp_ == 0),
                                stop=(pp_ == 3),
                            )
                        nc.scalar.copy(
                            y_sb[:, half * 512:(half + 1) * 512], yq[:])
                    nc.sync.dma_start(y_d[i * 128:(i + 1) * 128, :], y_sb[:])

            stream = [(qb, hh) for qb in range(3) for hh in range(HL)]
            ptbs = {}
            op_pending = []  # (qb, countdown)

            def drain_mm(idx):
                k = idx - LAG
                if 0 <= k < len(stream):
                    qbk, hhk = stream[k]
                    return emit_pv_mm(qbk, hhk, ptbs.pop((qbk, hhk)))
                return None

            def drain_norm(idx, ot_ps):
                k = idx - LAG
                if 0 <= k < len(stream):
                    qbk, hhk = stream[k]
                    emit_pv_norm(qbk, hhk, ot_ps)
                    if hhk == HL - 1:
                        op_pending.append([qbk, 2])
                for ent in list(op_pending):
                    ent[1] -= 1
                    if ent[1] <= 0:
                        emit_op(ent[0])
                        op_pending.remove(ent)

            for idx, (qb, hh) in enumerate(stream):
                if qb == 0 and hh == 2:
                    # V projection: after heads 0-1, before the first PV
                    for sc in range(NT):
                        proj_v(sc)
                if qb == 0 and hh in (2, 4, 6):
                    # projections for the dh tile these heads need
                    proj_qk(wq_d, qT, hh // 2)
                    proj_qk(wk_d, kT, hh // 2)
                nil = QBS[qb] // 128
                ptb = ptbp.tile([128, NT, 512], bf16, tag="ptb",
                                name=f"ptb{idx}")
                ptbs[(qb, hh)] = ptb
                # first ils of this head, then PV matmuls of the lagged head
                # (independent PE work between dependent QK tiles), then the
                # remaining ils, then the lagged head's normalize ops so the
                # DVE runs all four reduces back-to-back.
                for il in range(min(2, nil)):
                    emit_qk_il(qb, hh, il, ptb)
                ot_ps = drain_mm(idx)
                for il in range(2, nil):
                    emit_qk_il(qb, hh, il, ptb)
                drain_norm(idx, ot_ps)
            for idx in range(len(stream), len(stream) + LAG + 2):
                ot_ps = drain_mm(idx)
                drain_norm(idx, ot_ps)

    nc.compile()
    return nc


def _prep_inputs(x, mask, WQ, WK, WV, WO):
    idx_list = [np.nonzero(mask[b])[0] for b in range(B)]
    in_maps = []
    for c in range(8):
        b, g = c // 2, c % 2
        idx = idx_list[b]
        xc = np.zeros((SV, D), np.float32)
        xc[:len(idx)] = x[b][idx]
        perm = np.array(
            [dk * H + (g * HL + hh) for hh in range(HL) for dk in range(DK)]
        )
        in_maps.append({
            "xT": np.ascontiguousarray(xc.T),
            "wq": np.ascontiguousarray(WQ[:, perm] / np.sqrt(DK)).astype(np.float32),
            "wk": np.ascontiguousarray(WK[:, perm]).astype(np.float32),
            "wv": np.ascontiguousarray(WV[:, perm]).astype(np.float32),
            "wo": np.ascontiguousarray(WO[g * DH:(g + 1) * DH, :]).astype(np.float32),
        })
    return in_maps, idx_list


def _ref_fallback(x, mask, WQ, WK, WV, WO):
    # numpy fallback for masks with > SVC valid rows in a batch (never the
    # case for the target distribution); keeps kernel() correct for any mask.
    out = np.empty((B, S, D), np.float32)
    for b in range(B):
        q = (x[b] @ WQ).reshape(S, DK, H).transpose(2, 0, 1)
        k = (x[b] @ WK).reshape(S, DK, H).transpose(2, 1, 0)
        s = (q @ k) / np.sqrt(DK) - (~mask[b]).astype(np.float32)[None, None, :] * 1e6
        s = s - s.max(axis=-1, keepdims=True)
        e = np.exp(s)
        p = e / e.sum(axis=-1, keepdims=True)
        v = (x[b] @ WV).reshape(S, DK, H).transpose(2, 0, 1)
        o = (p @ v).transpose(1, 0, 2).reshape(S, D)
        out[b] = np.abs((o @ WO) * mask[b].astype(np.float32)[:, None])
    return out


def kernel(x, mask, WQ, WK, WV, WO, _want_results=False, _trace=False):
    from concourse.bass_utils import run_bass_kernel_spmd

    x = np.asarray(x, dtype=np.float32)
    mask = np.asarray(mask).astype(bool)
    WQ, WK = np.asarray(WQ, np.float32), np.asarray(WK, np.float32)
    WV, WO = np.asarray(WV, np.float32), np.asarray(WO, np.float32)

    if max(int(mask[b].sum()) for b in range(B)) > SVC:
        return _ref_fallback(x, mask, WQ, WK, WV, WO)

    if "nc" not in _cache:
        _cache["nc"] = _build()
    nc = _cache["nc"]
    in_maps, idx_list = _prep_inputs(x, mask, WQ, WK, WV, WO)
    res = run_bass_kernel_spmd(nc, in_maps, list(range(8)), trace=_trace)
    out = np.zeros((B, S, D), np.float32)
    for b in range(B):
        idx = idx_list[b]
        yb = res.results[2 * b]["y"][:len(idx)] + res.results[2 * b + 1]["y"][:len(idx)]
        out[b][idx] = np.abs(yb)
    if _want_results:
        return out, res
    return out


# revision 35
# speedup vs baseline: 1.0500x; 1.0500x over previous
"""TRN2 Bass kernel: MultiHeadSelfAttention (B=4, S=2048, D=1024, H=16, DK=64).

Sharding: 8 cores = 4 batches x 2 head-groups (8 heads each).

Key optimization vs the dense version: the padding mask kills ~half the keys
(exp(-1e6) == 0 exactly in f32) and ~half the queries (output is multiplied
by the query mask), so the host compacts each batch to its valid rows
(max 1044 for this distribution) padded to SV=1152. All attention work
(QK, softmax, PV) shrinks ~3.2x and the projections ~1.8x, exactly.

Per core: QK in f32r (TF32), softmax via one wide reduce_max (negated) +
one wide exp(bias=-max) -> bf16 P, P^T via DMA-transpose, PV with [V|1]
stationary -> [O^T; denom], 1/denom broadcast, normalization on gpsimd,
output projection from O^T, partial Y out. Host sums the two head-group
partials, applies abs, and scatters to valid positions.

The (qb, head) stream is software-pipelined: PV/output-projection for
head j runs while QK/softmax for head j+LAG streams, so PE never waits
on the DVE->Act->DMA softmax chain.
"""

import os
import numpy as np

B, S, D, H, DK = 4, 2048, 1024, 16, 64
HG = 2            # head groups (tensor-parallel)
HL = H // HG      # heads per core = 8
DH = HL * DK      # 512 per-core head width
KT = D // 128     # 8 contraction tiles
SV = 1152         # padded valid-row count (max valid ~1044 for this dist)
SVC = 1056        # trimmed compute width (>= max valid count, mult of 32)
NT = SV // 128    # 9 tiles of 128 (q tiles == k chunks)
QBS = (512, 512, 128)   # q block widths (sum == SV)
QBC = (512, 512, 32)    # q block valid widths (sum == SVC)
LAG = 3           # software pipeline depth in heads

_cache = {}


def _build():
    from concourse import bacc
    import concourse.mybir as mybir
    import concourse.tile as tile

    f32 = mybir.dt.float32
    f32r = mybir.dt.float32r
    bf16 = mybir.dt.bfloat16
    Exp = mybir.ActivationFunctionType.Exp
    AX = mybir.AxisListType.X

    nc = bacc.Bacc("TRN2", target_bir_lowering=False, debug=False, num_devices=8)

    xT_d = nc.dram_tensor("xT", [D, SV], f32, kind="ExternalInput")
    wq_d = nc.dram_tensor("wq", [D, DH], f32, kind="ExternalInput")
    wk_d = nc.dram_tensor("wk", [D, DH], f32, kind="ExternalInput")
    wv_d = nc.dram_tensor("wv", [D, DH], f32, kind="ExternalInput")
    wo_d = nc.dram_tensor("wo", [DH, D], f32, kind="ExternalInput")
    y_d = nc.dram_tensor("y", [SV, D], f32, kind="ExternalOutput")

    with tile.TileContext(nc) as tc:
        with (
            tc.tile_pool(name="persist", bufs=1) as pp,
            tc.tile_pool(name="psS", bufs=2, space="PSUM") as psS,
            tc.tile_pool(name="psSm", bufs=2, space="PSUM") as psSm,
            tc.tile_pool(name="ptbp", bufs=LAG + 1) as ptbp,
            tc.tile_pool(name="pexp", bufs=3) as pexp,
            tc.tile_pool(name="stats", bufs=2) as st,
            tc.tile_pool(name="oTp", bufs=2) as oTp,
            tc.tile_pool(name="yp", bufs=2) as yp,
            tc.tile_pool(name="ph1x", bufs=1) as px,
            tc.tile_pool(name="ph1w", bufs=2) as pw,
            tc.tile_pool(name="ph1wv", bufs=1) as pwv,
        ):
            qT = pp.tile([128, 4, SV], f32r, tag="qT")
            kT = pp.tile([128, 4, SV], f32r, tag="kT")
            # V with a ones column per head: blocks of 66 = [V_h(64) | 1 | pad]
            v_sb = pp.tile([128, NT, HL, 66], bf16, tag="v")
            nc.gpsimd.memset(v_sb[:, :, :, 64:65], 1.0)
            wor = pp.tile([128, 4, D], f32r, tag="wor")

            # ---- input loads ----
            xr = px.tile([128, KT, SV], f32r, tag="xr")
            nc.gpsimd.dma_start(xr[:], xT_d.rearrange("(t p) s -> p t s", p=128))
            wvr = pwv.tile([128, KT, DH], f32r, tag="wvr")
            wvr_loaded = [False]

            def load_wv_wo():
                nc.gpsimd.dma_start(
                    wvr[:], wv_d.rearrange("(t p) n -> p t n", p=128))
                nc.gpsimd.dma_start(
                    wor[:], wo_d.rearrange("(t p) n -> p t n", p=128))

            def proj_qk(w_d, dst, p):
                wchs = pw.tile([128, KT, 128], f32r, tag="wch")
                nc.gpsimd.dma_start(
                    wchs[:],
                    w_d[:, p * 128:(p + 1) * 128].rearrange(
                        "(t p) n -> p t n", p=128),
                )
                ps = psS.tile([128, SV], f32, tag="S")
                n0 = 0
                for nw in (512, 512, 32):
                    for k in range(KT):
                        nc.tensor.matmul(
                            ps[:, n0:n0 + nw],
                            wchs[:, k, :],
                            xr[:, k, n0:n0 + nw],
                            start=(k == 0),
                            stop=(k == KT - 1),
                        )
                    n0 += nw
                nc.vector.tensor_copy(dst[:, p, 0:SVC], ps[:, 0:SVC])

            def proj_v(sc):
                psv = psSm.tile([128, 512], f32, tag="mm")
                for k in range(KT):
                    nc.tensor.matmul(
                        psv[:],
                        xr[:, k, sc * 128:(sc + 1) * 128],
                        wvr[:, k, :],
                        start=(k == 0),
                        stop=(k == KT - 1),
                    )
                nc.scalar.copy(
                    v_sb[:, sc, :, 0:64],
                    psv[:].rearrange("p (h w) -> p h w", w=64),
                )

            # Q/K projections for dh tile 0 up front; tiles 1-3 are emitted
            # inside the attention stream right before the heads needing them
            proj_qk(wq_d, qT, 0)
            proj_qk(wk_d, kT, 0)
            load_wv_wo()

            # ---- attention stream, software-pipelined over (qb, head) ----
            qb_off = [0, 512, 1024]
            oTs = {}

            il_ctr = [0]

            def emit_qk_il(qb, hh, il, ptb):
                p, r0 = hh // 2, (hh % 2) * 64
                i = qb_off[qb] // 128 + il
                sq = psS.tile([128, SV], f32, tag="S")
                n0 = 0
                for nw in (512, 512, 32):
                    nc.tensor.matmul(
                        sq[:, n0:n0 + nw],
                        qT[r0:r0 + DK, p, i * 128:(i + 1) * 128],
                        kT[r0:r0 + DK, p, n0:n0 + nw],
                        start=True,
                        stop=True,
                    )
                    n0 += nw
                nm = st.tile([128, 1], f32, tag="nm")
                nc.vector.tensor_reduce(
                    nm[:], sq[:, 0:SVC], axis=AX,
                    op=mybir.AluOpType.max, negate=True,
                )
                il_ctr[0] += 1
                p_sb = pexp.tile([128, SV], bf16, tag="p")
                nc.scalar.activation(
                    p_sb[:, 0:SVC], sq[:, 0:SVC], Exp, bias=nm[:], scale=1.0,
                )
                nc.sync.dma_start(
                    ptb[:, :, il * 128:(il + 1) * 128],
                    p_sb[:],
                    transpose=True,
                )

            def emit_pv_mm(qb, hh, ptb):
                qw = QBC[qb]
                if hh == 0:
                    oTs[qb] = oTp.tile([128, 4, 512], f32r, tag="oT",
                                       name=f"oT{qb}")
                ot_ps = psSm.tile([65, 512], f32, tag="mm")
                for kc in range(NT):
                    kk = 128 if kc < NT - 1 else SVC - 128 * (NT - 1)
                    nc.tensor.matmul(
                        ot_ps[:, 0:qw],
                        v_sb[0:kk, kc, hh, 0:65],
                        ptb[0:kk, kc, 0:qw],
                        start=(kc == 0),
                        stop=(kc == NT - 1),
                    )
                return ot_ps

            def emit_pv_norm(qb, hh, ot_ps):
                p, r0 = hh // 2, (hh % 2) * 64
                qw = QBC[qb]
                rrow = st.tile([1, 512], f32, tag="rrow")
                nc.vector.reciprocal(rrow[:, 0:qw], ot_ps[64:65, 0:qw])
                rb = st.tile([64, 512], f32, tag="rb")
                nc.gpsimd.partition_broadcast(rb[:, 0:qw], rrow[:, 0:qw])
                osb = st.tile([64, 512], f32, tag="osb")
                nc.vector.tensor_copy(osb[:, 0:qw], ot_ps[0:64, 0:qw])
                nc.gpsimd.tensor_tensor(
                    oTs[qb][r0:r0 + 64, p, 0:qw], osb[:, 0:qw],
                    rb[:, 0:qw], op=mybir.AluOpType.mult,
                )

            def emit_op(qb):
                """Output projection for q block qb."""
                qw = QBS[qb]
                oT = oTs[qb]
                for il in range(qw // 128):
                    i = qb_off[qb] // 128 + il
                    y_sb = yp.tile([128, D], f32, tag="y")
                    for half in range(2):
                        yq = psSm.tile([128, 512], f32, tag="mm")
                        for pp_ in range(4):
                            nc.tensor.matmul(
                                yq[:],
                                oT[:, pp_, il * 128:(il + 1) * 128],
                                wor[:, pp_, half * 512:(half + 1) * 512],
                                start=(pp_ == 0),
                                stop=(pp_ == 3),
                            )
                        nc.scalar.copy(
                            y_sb[:, half * 512:(half + 1) * 512], yq[:])
                    nc.sync.dma_start(y_d[i * 128:(i + 1) * 128, :], y_sb[:])

            stream = [(qb, hh) for qb in range(3) for hh in range(HL)]
            ptbs = {}
            op_pending = []  # (qb, countdown)

            def drain_mm(idx):
                k = idx - LAG
                if 0 <= k < len(stream):
                    qbk, hhk = stream[k]
                    return emit_pv_mm(qbk, hhk, ptbs.pop((qbk, hhk)))
                return None

            def drain_norm(idx, ot_ps):
                k = idx - LAG
                if 0 <= k < len(stream):
                    qbk, hhk = stream[k]
                    emit_pv_norm(qbk, hhk, ot_ps)
                    if hhk == HL - 1:
                        op_pending.append([qbk, 2])
                for ent in list(op_pending):
                    ent[1] -= 1
                    if ent[1] <= 0:
                        emit_op(ent[0])
                        op_pending.remove(ent)

            for idx, (qb, hh) in enumerate(stream):
                if qb == 0 and hh == 2:
                    # V projection: after heads 0-1, before the first PV
                    for sc in range(NT):
                        proj_v(sc)
                if qb == 0 and hh in (2, 4, 6):
                    # projections for the dh tile these heads need
                    proj_qk(wq_d, qT, hh // 2)
                    proj_qk(wk_d, kT, hh // 2)
                nil = QBS[qb] // 128
                ptb = ptbp.tile([128, NT, 512], bf16, tag="ptb",
                                name=f"ptb{idx}")
                ptbs[(qb, hh)] = ptb
                # first ils of this head, then PV matmuls of the lagged head
                # (independent PE work between dependent QK tiles), then the
                # remaining ils, then the lagged head's normalize ops so the
                # DVE runs all four reduces back-to-back.
                for il in range(min(2, nil)):
                    emit_qk_il(qb, hh, il, ptb)
                ot_ps = drain_mm(idx)
                for il in range(2, nil):
                    emit_qk_il(qb, hh, il, ptb)
                drain_norm(idx, ot_ps)
            for idx in range(len(stream), len(stream) + LAG + 2):
                ot_ps = drain_mm(idx)
                drain_norm(idx, ot_ps)

    nc.compile()
    return nc


def _prep_inputs(x, mask, WQ, WK, WV, WO):
    idx_list = [np.nonzero(mask[b])[0] for b in range(B)]
    in_maps = []
    for c in range(8):
        b, g = c // 2, c % 2
        idx = idx_list[b]
        xc = np.zeros((SV, D), np.float32)
        xc[:len(idx)] = x[b][idx]
        perm = np.array(
            [dk * H + (g * HL + hh) for hh in range(HL) for dk in range(DK)]
        )
        in_maps.append({
            "xT": np.ascontiguousarray(xc.T),
            "wq": np.ascontiguousarray(WQ[:, perm] / np.sqrt(DK)).astype(np.float32),
            "wk": np.ascontiguousarray(WK[:, perm]).astype(np.float32),
            "wv": np.ascontiguousarray(WV[:, perm]).astype(np.float32),
            "wo": np.ascontiguousarray(WO[g * DH:(g + 1) * DH, :]).astype(np.float32),
        })
    return in_maps, idx_list


def _ref_fallback(x, mask, WQ, WK, WV, WO):
    # numpy fallback for masks with > SVC valid rows in a batch (never the
    # case for the target distribution); keeps kernel() correct for any mask.
    out = np.empty((B, S, D), np.float32)
    for b in range(B):
        q = (x[b] @ WQ).reshape(S, DK, H).transpose(2, 0, 1)
        k = (x[b] @ WK).reshape(S, DK, H).transpose(2, 1, 0)
        s = (q @ k) / np.sqrt(DK) - (~mask[b]).astype(np.float32)[None, None, :] * 1e6
        s = s - s.max(axis=-1, keepdims=True)
        e = np.exp(s)
        p = e / e.sum(axis=-1, keepdims=True)
        v = (x[b] @ WV).reshape(S, DK, H).transpose(2, 0, 1)
        o = (p @ v).transpose(1, 0, 2).reshape(S, D)
        out[b] = np.abs((o @ WO) * mask[b].astype(np.float32)[:, None])
    return out


def kernel(x, mask, WQ, WK, WV, WO, _want_results=False, _trace=False):
    from concourse.bass_utils import run_bass_kernel_spmd

    x = np.asarray(x, dtype=np.float32)
    mask = np.asarray(mask).astype(bool)
    WQ, WK = np.asarray(WQ, np.float32), np.asarray(WK, np.float32)
    WV, WO = np.asarray(WV, np.float32), np.asarray(WO, np.float32)

    if max(int(mask[b].sum()) for b in range(B)) > SVC:
        return _ref_fallback(x, mask, WQ, WK, WV, WO)

    if "nc" not in _cache:
        _cache["nc"] = _build()
    nc = _cache["nc"]
    in_maps, idx_list = _prep_inputs(x, mask, WQ, WK, WV, WO)
    res = run_bass_kernel_spmd(nc, in_maps, list(range(8)), trace=_trace)
    out = np.zeros((B, S, D), np.float32)
    for b in range(B):
        idx = idx_list[b]
        yb = res.results[2 * b]["y"][:len(idx)] + res.results[2 * b + 1]["y"][:len(idx)]
        out[b][idx] = np.abs(yb)
    if _want_results:
        return out, res
    return out


# revision 36
# speedup vs baseline: 1.0508x; 1.0008x over previous
"""TRN2 Bass kernel: MultiHeadSelfAttention (B=4, S=2048, D=1024, H=16, DK=64).

Sharding: 8 cores = 4 batches x 2 head-groups (8 heads each).

Key optimization vs the dense version: the padding mask kills ~half the keys
(exp(-1e6) == 0 exactly in f32) and ~half the queries (output is multiplied
by the query mask), so the host compacts each batch to its valid rows
(max 1044 for this distribution) padded to SV=1152. All attention work
(QK, softmax, PV) shrinks ~3.2x and the projections ~1.8x, exactly.

Per core: QK in f32r (TF32), softmax via one wide reduce_max (negated) +
one wide exp(bias=-max) -> bf16 P, P^T via DMA-transpose, PV with [V|1]
stationary -> [O^T; denom], 1/denom broadcast, normalization on gpsimd,
output projection from O^T, partial Y out. Host sums the two head-group
partials, applies abs, and scatters to valid positions.

The (qb, head) stream is software-pipelined: PV/output-projection for
head j runs while QK/softmax for head j+LAG streams, so PE never waits
on the DVE->Act->DMA softmax chain.
"""

import os
import numpy as np

B, S, D, H, DK = 4, 2048, 1024, 16, 64
HG = 2            # head groups (tensor-parallel)
HL = H // HG      # heads per core = 8
DH = HL * DK      # 512 per-core head width
KT = D // 128     # 8 contraction tiles
SV = 1152         # padded valid-row count (max valid ~1044 for this dist)
SVC = 1056        # trimmed compute width (>= max valid count, mult of 32)
NT = SV // 128    # 9 tiles of 128 (q tiles == k chunks)
QBS = (512, 512, 128)   # q block widths (sum == SV)
QBC = (512, 512, 32)    # q block valid widths (sum == SVC)
LAG = 3           # software pipeline depth in heads

_cache = {}


def _build():
    from concourse import bacc
    import concourse.mybir as mybir
    import concourse.tile as tile

    f32 = mybir.dt.float32
    f32r = mybir.dt.float32r
    bf16 = mybir.dt.bfloat16
    Exp = mybir.ActivationFunctionType.Exp
    AX = mybir.AxisListType.X

    nc = bacc.Bacc("TRN2", target_bir_lowering=False, debug=False, num_devices=8)

    xT_d = nc.dram_tensor("xT", [D, SV], f32, kind="ExternalInput")
    wq_d = nc.dram_tensor("wq", [D, DH], f32, kind="ExternalInput")
    wk_d = nc.dram_tensor("wk", [D, DH], f32, kind="ExternalInput")
    wv_d = nc.dram_tensor("wv", [D, DH], f32, kind="ExternalInput")
    wo_d = nc.dram_tensor("wo", [DH, D], f32, kind="ExternalInput")
    y_d = nc.dram_tensor("y", [SV, D], f32, kind="ExternalOutput")

    with tile.TileContext(nc) as tc:
        with (
            tc.tile_pool(name="persist", bufs=1) as pp,
            tc.tile_pool(name="psS", bufs=2, space="PSUM") as psS,
            tc.tile_pool(name="psSm", bufs=2, space="PSUM") as psSm,
            tc.tile_pool(name="ptbp", bufs=LAG + 1) as ptbp,
            tc.tile_pool(name="pexp", bufs=3) as pexp,
            tc.tile_pool(name="stats", bufs=3) as st,
            tc.tile_pool(name="oTp", bufs=2) as oTp,
            tc.tile_pool(name="yp", bufs=2) as yp,
            tc.tile_pool(name="ph1x", bufs=1) as px,
            tc.tile_pool(name="ph1w", bufs=2) as pw,
            tc.tile_pool(name="ph1wv", bufs=1) as pwv,
        ):
            qT = pp.tile([128, 4, SV], f32r, tag="qT")
            kT = pp.tile([128, 4, SV], f32r, tag="kT")
            # V with a ones column per head: blocks of 66 = [V_h(64) | 1 | pad]
            v_sb = pp.tile([128, NT, HL, 66], bf16, tag="v")
            nc.gpsimd.memset(v_sb[:, :, :, 64:65], 1.0)
            wor = pp.tile([128, 4, D], f32r, tag="wor")

            # ---- input loads ----
            xr = px.tile([128, KT, SV], f32r, tag="xr")
            nc.gpsimd.dma_start(xr[:], xT_d.rearrange("(t p) s -> p t s", p=128))
            wvr = pwv.tile([128, KT, DH], f32r, tag="wvr")
            wvr_loaded = [False]

            def load_wv_wo():
                nc.gpsimd.dma_start(
                    wvr[:], wv_d.rearrange("(t p) n -> p t n", p=128))
                nc.gpsimd.dma_start(
                    wor[:], wo_d.rearrange("(t p) n -> p t n", p=128))

            def proj_qk(w_d, dst, p):
                wchs = pw.tile([128, KT, 128], f32r, tag="wch")
                nc.gpsimd.dma_start(
                    wchs[:],
                    w_d[:, p * 128:(p + 1) * 128].rearrange(
                        "(t p) n -> p t n", p=128),
                )
                ps = psS.tile([128, SV], f32, tag="S")
                n0 = 0
                for nw in (512, 512, 32):
                    for k in range(KT):
                        nc.tensor.matmul(
                            ps[:, n0:n0 + nw],
                            wchs[:, k, :],
                            xr[:, k, n0:n0 + nw],
                            start=(k == 0),
                            stop=(k == KT - 1),
                        )
                    n0 += nw
                nc.vector.tensor_copy(dst[:, p, 0:SVC], ps[:, 0:SVC])

            def proj_v(sc):
                psv = psSm.tile([128, 512], f32, tag="mm")
                for k in range(KT):
                    nc.tensor.matmul(
                        psv[:],
                        xr[:, k, sc * 128:(sc + 1) * 128],
                        wvr[:, k, :],
                        start=(k == 0),
                        stop=(k == KT - 1),
                    )
                nc.scalar.copy(
                    v_sb[:, sc, :, 0:64],
                    psv[:].rearrange("p (h w) -> p h w", w=64),
                )

            # Q/K projections for dh tile 0 up front; tiles 1-3 are emitted
            # inside the attention stream right before the heads needing them
            proj_qk(wq_d, qT, 0)
            proj_qk(wk_d, kT, 0)
            load_wv_wo()

            # ---- attention stream, software-pipelined over (qb, head) ----
            qb_off = [0, 512, 1024]
            oTs = {}

            il_ctr = [0]

            def emit_qk_il(qb, hh, il, ptb):
                p, r0 = hh // 2, (hh % 2) * 64
                i = qb_off[qb] // 128 + il
                sq = psS.tile([128, SV], f32, tag="S")
                n0 = 0
                for nw in (512, 512, 32):
                    nc.tensor.matmul(
                        sq[:, n0:n0 + nw],
                        qT[r0:r0 + DK, p, i * 128:(i + 1) * 128],
                        kT[r0:r0 + DK, p, n0:n0 + nw],
                        start=True,
                        stop=True,
                    )
                    n0 += nw
                nm = st.tile([128, 1], f32, tag="nm")
                nc.vector.tensor_reduce(
                    nm[:], sq[:, 0:SVC], axis=AX,
                    op=mybir.AluOpType.max, negate=True,
                )
                il_ctr[0] += 1
                p_sb = pexp.tile([128, SV], bf16, tag="p")
                nc.scalar.activation(
                    p_sb[:, 0:SVC], sq[:, 0:SVC], Exp, bias=nm[:], scale=1.0,
                )
                nc.sync.dma_start(
                    ptb[:, :, il * 128:(il + 1) * 128],
                    p_sb[:],
                    transpose=True,
                )

            def emit_pv_mm(qb, hh, ptb):
                qw = QBC[qb]
                if hh == 0:
                    oTs[qb] = oTp.tile([128, 4, 512], f32r, tag="oT",
                                       name=f"oT{qb}")
                ot_ps = psSm.tile([65, 512], f32, tag="mm")
                for kc in range(NT):
                    kk = 128 if kc < NT - 1 else SVC - 128 * (NT - 1)
                    nc.tensor.matmul(
                        ot_ps[:, 0:qw],
                        v_sb[0:kk, kc, hh, 0:65],
                        ptb[0:kk, kc, 0:qw],
                        start=(kc == 0),
                        stop=(kc == NT - 1),
                    )
                return ot_ps

            def emit_pv_norm(qb, hh, ot_ps):
                p, r0 = hh // 2, (hh % 2) * 64
                qw = QBC[qb]
                rrow = st.tile([1, 512], f32, tag="rrow")
                nc.vector.reciprocal(rrow[:, 0:qw], ot_ps[64:65, 0:qw])
                rb = st.tile([64, 512], f32, tag="rb")
                nc.gpsimd.partition_broadcast(rb[:, 0:qw], rrow[:, 0:qw])
                osb = st.tile([64, 512], f32, tag="osb")
                nc.vector.tensor_copy(osb[:, 0:qw], ot_ps[0:64, 0:qw])
                nc.gpsimd.tensor_tensor(
                    oTs[qb][r0:r0 + 64, p, 0:qw], osb[:, 0:qw],
                    rb[:, 0:qw], op=mybir.AluOpType.mult,
                )

            def emit_op(qb):
                """Output projection for q block qb."""
                qw = QBS[qb]
                oT = oTs[qb]
                for il in range(qw // 128):
                    i = qb_off[qb] // 128 + il
                    y_sb = yp.tile([128, D], f32, tag="y")
                    for half in range(2):
                        yq = psSm.tile([128, 512], f32, tag="mm")
                        for pp_ in range(4):
                            nc.tensor.matmul(
                                yq[:],
                                oT[:, pp_, il * 128:(il + 1) * 128],
                                wor[:, pp_, half * 512:(half + 1) * 512],
                                start=(pp_ == 0),
                                stop=(pp_ == 3),
                            )
                        nc.scalar.copy(
                            y_sb[:, half * 512:(half + 1) * 512], yq[:])
                    nc.sync.dma_start(y_d[i * 128:(i + 1) * 128, :], y_sb[:])

            stream = [(qb, hh) for qb in range(3) for hh in range(HL)]
            ptbs = {}
            op_pending = []  # (qb, countdown)

            def drain_mm(idx):
                k = idx - LAG
                if 0 <= k < len(stream):
                    qbk, hhk = stream[k]
                    return emit_pv_mm(qbk, hhk, ptbs.pop((qbk, hhk)))
                return None

            def drain_norm(idx, ot_ps):
                k = idx - LAG
                if 0 <= k < len(stream):
                    qbk, hhk = stream[k]
                    emit_pv_norm(qbk, hhk, ot_ps)
                    if hhk == HL - 1:
                        op_pending.append([qbk, 2])
                for ent in list(op_pending):
                    ent[1] -= 1
                    if ent[1] <= 0:
                        emit_op(ent[0])
                        op_pending.remove(ent)

            for idx, (qb, hh) in enumerate(stream):
                if qb == 0 and hh == 2:
                    # V projection: after heads 0-1, before the first PV
                    for sc in range(NT):
                        proj_v(sc)
                if qb == 0 and hh in (2, 4, 6):
                    # projections for the dh tile these heads need
                    proj_qk(wq_d, qT, hh // 2)
                    proj_qk(wk_d, kT, hh // 2)
                nil = QBS[qb] // 128
                ptb = ptbp.tile([128, NT, 512], bf16, tag="ptb",
                                name=f"ptb{idx}")
                ptbs[(qb, hh)] = ptb
                # first ils of this head, then PV matmuls of the lagged head
                # (independent PE work between dependent QK tiles), then the
                # remaining ils, then the lagged head's normalize ops so the
                # DVE runs all four reduces back-to-back.
                for il in range(min(2, nil)):
                    emit_qk_il(qb, hh, il, ptb)
                ot_ps = drain_mm(idx)
                for il in range(2, nil):
                    emit_qk_il(qb, hh, il, ptb)
                drain_norm(idx, ot_ps)
            for idx in range(len(stream), len(stream) + LAG + 2):
                ot_ps = drain_mm(idx)
                drain_norm(idx, ot_ps)

    nc.compile()
    return nc


def _prep_inputs(x, mask, WQ, WK, WV, WO):
    idx_list = [np.nonzero(mask[b])[0] for b in range(B)]
    in_maps = []
    for c in range(8):
        b, g = c // 2, c % 2
        idx = idx_list[b]
        xc = np.zeros((SV, D), np.float32)
        xc[:len(idx)] = x[b][idx]
        perm = np.array(
            [dk * H + (g * HL + hh) for hh in range(HL) for dk in range(DK)]
        )
        in_maps.append({
            "xT": np.ascontiguousarray(xc.T),
            "wq": np.ascontiguousarray(WQ[:, perm] / np.sqrt(DK)).astype(np.float32),
            "wk": np.ascontiguousarray(WK[:, perm]).astype(np.float32),
            "wv": np.ascontiguousarray(WV[:, perm]).astype(np.float32),
            "wo": np.ascontiguousarray(WO[g * DH:(g + 1) * DH, :]).astype(np.float32),
        })
    return in_maps, idx_list


def _ref_fallback(x, mask, WQ, WK, WV, WO):
    # numpy fallback for masks with > SVC valid rows in a batch (never the
    # case for the target distribution); keeps kernel() correct for any mask.
    out = np.empty((B, S, D), np.float32)
    for b in range(B):
        q = (x[b] @ WQ).reshape(S, DK, H).transpose(2, 0, 1)
        k = (x[b] @ WK).reshape(S, DK, H).transpose(2, 1, 0)
        s = (q @ k) / np.sqrt(DK) - (~mask[b]).astype(np.float32)[None, None, :] * 1e6
        s = s - s.max(axis=-1, keepdims=True)
        e = np.exp(s)
        p = e / e.sum(axis=-1, keepdims=True)
        v = (x[b] @ WV).reshape(S, DK, H).transpose(2, 0, 1)
        o = (p @ v).transpose(1, 0, 2).reshape(S, D)
        out[b] = np.abs((o @ WO) * mask[b].astype(np.float32)[:, None])
    return out


def kernel(x, mask, WQ, WK, WV, WO, _want_results=False, _trace=False):
    from concourse.bass_utils import run_bass_kernel_spmd

    x = np.asarray(x, dtype=np.float32)
    mask = np.asarray(mask).astype(bool)
    WQ, WK = np.asarray(WQ, np.float32), np.asarray(WK, np.float32)
    WV, WO = np.asarray(WV, np.float32), np.asarray(WO, np.float32)

    if max(int(mask[b].sum()) for b in range(B)) > SVC:
        return _ref_fallback(x, mask, WQ, WK, WV, WO)

    if "nc" not in _cache:
        _cache["nc"] = _build()
    nc = _cache["nc"]
    in_maps, idx_list = _prep_inputs(x, mask, WQ, WK, WV, WO)
    res = run_bass_kernel_spmd(nc, in_maps, list(range(8)), trace=_trace)
    out = np.zeros((B, S, D), np.float32)
    for b in range(B):
        idx = idx_list[b]
        yb = res.results[2 * b]["y"][:len(idx)] + res.results[2 * b + 1]["y"][:len(idx)]
        out[b][idx] = np.abs(yb)
    if _want_results:
        return out, res
    return out
